# revision 42
# baseline (speedup 1.0000x reference)
"""Trainium2 Bass kernel for nn_MOTASG_KO_Reg (ragged graph-conv KO regression).

Strategy (8 NeuronCores, data-parallel over node rows):
  - N=16384 nodes = 16 batch samples x 1024 entities. Core c owns rows
    [2048c, 2048c+2048) = batch samples 2c, 2c+1.
  - Weight fusion: m1 = cross_c @ ienc_W is computed directly as
    om_act @ (fus_om_pad @ ienc_W) + nd_act @ (fus_nd_pad @ ienc_W), so the
    cross tensor is never materialized. The ko column routes through om_act
    row 511 x W_comb row 511 (= ienc_W[511]).
  - nd path sharded 128 entities/core -> m1_nd [128,512] row-major,
    AllGathered (issued first on the cc stream), added into m1 tiles.
  - m1 tiles stay in SBUF (self term) and are stashed to DRAM for ONE
    AllGather; gconv1 neighbor sums use dma_gather (single_packet) of
    AllGathered m1 rows + one-hot PE scatter into per-dst-tile PSUM.
  - z2 = x_c + pre@pre_W + lrelu(gconv1) accumulated in place on xt;
    m2 = z2 @ enc_W computed once (no m2a split), tiles stay in SBUF.
  - gconv2 (only 1024 KO slots needed): block-dense count matrices
    A2[slot_tile, row_tile] [128,128] -> 128 PE matmuls over SBUF m2 tiles;
    no gather, no DRAM round trip. ONE fp16 ReduceScatter -> 128 slots/core.
  - Readout (gate + softmax + weighted sum + regression) on-core -> [2].
"""

import functools
import numpy as np
import ml_dtypes

import concourse.bacc as bacc
import concourse.mybir as mybir
import concourse.tile as tile
from concourse import bass
from concourse.bass_utils import run_bass_kernel_spmd
from concourse.masks import make_identity

NE, B, KO = 1024, 16, 64
TX, OM, D = 768, 511, 512
N = NE * B
NCORE = 8
R = N // NCORE        # 2048 rows per core
NT = R // 128         # 16 row tiles per core
SLOPE = 0.3
F32 = mybir.dt.float32
F16 = mybir.dt.float16
F8 = mybir.dt.float8e4
I16 = mybir.dt.int16
AX = mybir.AxisListType.X
ALU = mybir.AluOpType
ACTF = mybir.ActivationFunctionType

WAVE = 8  # gather chunks per dma_gather call
WCOLS = WAVE * 8
FP8_AG = True         # AllGather m1 + gather payload in fp8 (scaled)
AG_SCALE = 64.0   # descale folded into sel (= 1/64)
H = R // 2            # AG1 row-half (AG1 split into 2 pipelined collectives)
DEBUG = False
TRACE = False
TRACE_KW = None


# ---------------------------------------------------------------------------
# host-side edge preparation
# ---------------------------------------------------------------------------

def _chunk_edges_per_tile(src, dstl, nch_t):
    """Sort (src->dst_local) into per-destination-tile 128-edge chunks."""
    C = sum(nch_t)
    idx = np.zeros((C, 128), np.int16)
    dstv = np.full((C, 128), -2.0, np.float32)
    t_of = dstl >> 7
    base = 0
    for t, nch in enumerate(nch_t):
        m = t_of == t
        s = src[m]
        d = (dstl[m] - (t << 7)).astype(np.float32)
        n = len(s)
        assert n <= nch * 128, (n, nch)
        full, rem = divmod(n, 128)
        for j in range(full):
            idx[base + j] = s[j * 128:(j + 1) * 128]
            dstv[base + j] = d[j * 128:(j + 1) * 128]
        if rem:
            idx[base + full, :rem] = s[full * 128:]
            dstv[base + full, :rem] = d[full * 128:]
        base += nch
    return idx, dstv


def _wrap_idx_waves(idx_chunks):
    """[C,128] int16 -> [128, nwaves*128] wrapped per dma_gather call."""
    C = idx_chunks.shape[0]
    cols = []
    for w in range((C + WAVE - 1) // WAVE):
        lin = idx_chunks[w * WAVE:(w + 1) * WAVE].reshape(-1)
        wrapped = lin.reshape(-1, 16).T
        pad = np.zeros((16, WAVE * 8 - wrapped.shape[1]), np.int16)
        cols.append(np.tile(np.concatenate([wrapped, pad], 1), (8, 1)))
    return np.ascontiguousarray(np.concatenate(cols, axis=1))


def _sel_from_dstv(dstv, dt, val=1.0):
    C = dstv.shape[0]
    sel = (dstv[:, :, None] == np.arange(128, dtype=np.float32)[None, None, :])
    return np.ascontiguousarray((sel * val).reshape(C * 128, 128).astype(dt))


def _pad_w(w, rows, cols):
    out = np.zeros((rows, cols), np.float32)
    out[:w.shape[0], :w.shape[1]] = w
    return out


# ---------------------------------------------------------------------------
# program builder
# ---------------------------------------------------------------------------

@functools.lru_cache(maxsize=4)
def _build(nch1_t):
    """nch1_t: gconv1 chunks per dst tile (len 16); total multiple of WAVE."""
    C1 = sum(nch1_t)
    W1 = C1 // WAVE
    GDT = F8 if FP8_AG else F16
    nc = bacc.Bacc("TRN2", num_swdge_queues=4)

    def din(name, shape, dtype=F16):
        return nc.dram_tensor(name, shape, dtype, kind="ExternalInput")

    x_t = din("x_t", [512, R])                  # [x | ko]^T fp16
    pre_t_d = din("pre_t", [512, R])
    ndemb = din("ndemb", [2 * TX, 128])
    name_W = din("name_W", [TX, TX])
    desc_W = din("desc_W", [TX, TX])
    wnd_comb = din("wnd_comb", [2 * TX, 512])   # pad(fus_nd) @ ienc_W
    omic_W = din("omic_W", [512, 512])
    w_comb = din("w_comb", [512, 512])          # pad(fus_om) @ ienc_W (+ko row)
    pre_W = din("pre_W", [512, 512])
    enc_W = din("enc_W", [512, 512])
    gate_W1 = din("gate_W1", [512, 512], F16)
    gw2reg = din("gw2reg", [128, 8], F16)
    bias_pf = din("bias_pf", [128, 26], F32)
    bias_rows = din("bias_rows", [96, 512], F16)
    idx1_d = din("idx1", [128, W1 * WCOLS], I16)
    sel1_d = din("sel1", [C1 * 128, 128], GDT)
    a2_d = din("a2", [128 * 128, 128])          # [s*16+t][128 rows,128 slots]
    ae_d = din("ae", [16 * 8 * 128, 128], GDT)  # [t*8+e][128 ent,128 dst] /S
    out_d = nc.dram_tensor("out", [1, 2], F32, kind="ExternalOutput")

    agnd_in = nc.dram_tensor("agnd_in", [128, 512], F16)
    agnd_out = nc.dram_tensor("agnd_out", [NCORE * 128, 512], F16,
                              addr_space="Shared")
    ag1_in = nc.dram_tensor("ag1_in", [R, 512], GDT)
    # two half AllGathers write [half][core][1024 rows]; gather indices are
    # host-remapped into this concatenated layout
    ag1_out = nc.dram_tensor("ag1_out", [N, 512], GDT, addr_space="Shared")
    rs_in = nc.dram_tensor("rs_in", [8 * 128, 512], F16)
    rs_out = nc.dram_tensor("rs_out", [128, 512], F16)
    RG = [list(range(NCORE))]

    if DEBUG:
        dbg_m1 = nc.dram_tensor("dbg_m1", [R, 512], GDT, kind="ExternalOutput")
        dbg_m2 = nc.dram_tensor("dbg_m2", [R, 512], F16, kind="ExternalOutput")
        dbg_zk = nc.dram_tensor("dbg_zk", [128, 512], F16, kind="ExternalOutput")

    with tile.TileContext(nc) as tc:
        with (
            tc.tile_pool(name="pbig", bufs=4) as pbig,
            tc.tile_pool(name="pmed", bufs=1) as pmed,
            tc.tile_pool(name="pkeep", bufs=1) as pkeep,
            tc.tile_pool(name="pw", bufs=1) as pw,
            tc.tile_pool(name="pg", bufs=1) as pg,
            tc.tile_pool(name="psc", bufs=1) as psc,
            tc.tile_pool(name="pp", bufs=1, space="PSUM") as pp,
        ):
            # ---- constants ----
            bpf = psc.tile([128, 26], F32, tag="bpf", bufs=1)
            nc.sync.dma_start(out=bpf[:], in_=bias_pf[:])
            brow_g = psc.tile([1, 512], F16, tag="brow_g", bufs=1)
            nc.sync.dma_start(out=brow_g[:], in_=bias_rows[64:65, :])
            ones = psc.tile([1, 512], F16, tag="ones", bufs=1)
            nc.vector.memset(ones[:], 1.0)
            ident = psc.tile([128, 128], F32, tag="ident", bufs=1)
            make_identity(nc, ident[:])
            ident16 = psc.tile([128, 128], F16, tag="ident16", bufs=1)
            make_identity(nc, ident16[:])
            idx1 = psc.tile([128, W1 * WCOLS], I16, tag="idx1", bufs=1)
            nc.sync.dma_start(out=idx1[:], in_=idx1_d[:])

            # ---- big activations (fp16) ----
            xt = []
            for k in range(4):
                t = pbig.tile([128, R], F16, tag="bigA", bufs=4, name=f"xt{k}")
                nc.sync.dma_start(out=t[:], in_=x_t[128 * k:128 * (k + 1), :])
                xt.append(t)

            # ---- prep-ahead gather waves: descriptors generated now (gpsimd
            # is idle), DMAs fired by trigger_dma after AG1 completes ----
            PREPW = min(0, W1)
            gbufs = {}
            for w in range(PREPW):
                g = pg.tile([128, WAVE, 512], GDT, tag="gath", bufs=4)
                psem = nc.alloc_semaphore(f"g1dma{w}")
                nc.gpsimd.dma_gather(
                    g[:, :, :], ag1_out[:],
                    idx1[:, WCOLS * w:WCOLS * w + WAVE * 8],
                    WAVE * 128, WAVE * 128, 512,
                    single_packet=True, prepare_only=True, sem=psem,
                    queue_num=1 + w)
                gbufs[w] = g

            # ---- om pass + m1_om, interleaved per j ----
            womic = [pw.tile([128, 512], F16, tag="wres", bufs=16, name=f"womic{k}")
                     for k in range(4)]
            wcomb = [pw.tile([128, 512], F16, tag="wres", bufs=16, name=f"wcomb{k}")
                     for k in range(4)]
            for k in range(4):
                nc.sync.dma_start(out=womic[k][:], in_=omic_W[128 * k:128 * (k + 1), :])
                nc.sync.dma_start(out=wcomb[k][:], in_=w_comb[128 * k:128 * (k + 1), :])
            m1sb = [pkeep.tile([128, 512], F16, tag="m1keep", bufs=16,
                               name=f"m1sb{t}") for t in range(NT)]
            om_all = []

            def emit_m1om(j):
                om_j = om_all[j]
                for tt in range(4):
                    t = 4 * j + tt
                    tsl = slice(128 * tt, 128 * (tt + 1))
                    ps = pp.tile([128, 512], F32, tag="ps_mm", bufs=2, space="PSUM")
                    for ki in range(4):
                        nc.tensor.matmul(ps[:], lhsT=om_j[ki][:, tsl], rhs=wcomb[ki][:],
                                         start=(ki == 0), stop=(ki == 3))
                    nc.scalar.activation(m1sb[t][:], ps[:], ACTF.Copy)

            for j in range(4):
                sl = slice(512 * j, 512 * (j + 1))
                om_j = []
                for k in range(4):
                    ps = pp.tile([128, 512], F32, tag="ps_mm", bufs=2, space="PSUM")
                    for ki in range(4):
                        nc.tensor.matmul(ps[:], lhsT=womic[ki][:, 128 * k:128 * (k + 1)],
                                         rhs=xt[ki][:, sl], start=(ki == 0), stop=(ki == 3))
                    a = pmed.tile([128, 512], F16, tag="omj", bufs=8)
                    tom = pmed.tile([128, 512], F32, tag="tmpom", bufs=2)
                    nc.vector.tensor_scalar(out=tom[:], in0=ps[:], scalar1=SLOPE,
                                            scalar2=None, op0=ALU.mult)
                    nc.vector.tensor_tensor(out=a[:], in0=ps[:], in1=tom[:], op=ALU.max)
                    om_j.append(a)
                # ko feature (col 511) = om row 127 of k=3 block
                nc.sync.dma_start(out=om_j[3][127:128, :], in_=x_t[511:512, sl])
                om_all.append(om_j)
                if j >= 1:
                    emit_m1om(j - 1)
            emit_m1om(3)

            # ---- stash om-only m1 for AG1 (nd folded in via A_e matmuls) ----
            for t in range(NT):
                if FP8_AG:
                    h8 = pmed.tile([128, 512], F8, tag="m1f8", bufs=3)
                    nc.scalar.activation(h8[:], m1sb[t][:], ACTF.Copy,
                                         scale=AG_SCALE)
                    nc.sync.dma_start(out=ag1_in[128 * t:128 * (t + 1), :], in_=h8[:])
                else:
                    nc.sync.dma_start(out=ag1_in[128 * t:128 * (t + 1), :],
                                      in_=m1sb[t][:])
            nc.gpsimd.collective_compute(
                "AllGather", ALU.bypass, replica_groups=RG,
                ins=[ag1_in[:]], outs=[ag1_out[:]])
            for w in range(PREPW):
                nc.gpsimd.trigger_dma(count=None, queue_num=1 + w)
            if DEBUG:
                nc.sync.dma_start(out=dbg_m1[:], in_=ag1_in[:])

            # ---- ND path (128 entities) -> m1_nd row-major [128, 512] ----
            nd_act = []
            for half in range(2):
                W_d = name_W if half == 0 else desc_W
                embs = []
                for ki in range(6):
                    e_ = psc.tile([128, 128], F16, tag="emb", bufs=7,
                                  name=f"emb{half}_{ki}")
                    nc.sync.dma_start(
                        out=e_[:],
                        in_=ndemb[half * TX + 128 * ki: half * TX + 128 * (ki + 1), :])
                    embs.append(e_)
                for mo in range(6):
                    ps = pp.tile([128, 512], F32, tag="ps_mm", bufs=2, space="PSUM")
                    wstrip = pw.tile([128, 6, 128], F16, tag="wnd6", bufs=3)
                    nc.sync.dma_start(
                        out=wstrip[:],
                        in_=W_d[:, 128 * mo:128 * (mo + 1)].rearrange(
                            "(ki p) m -> p ki m", p=128))
                    for ki in range(6):
                        nc.tensor.matmul(ps[:, :128], lhsT=wstrip[:, ki, :],
                                         rhs=embs[ki][:],
                                         start=(ki == 0), stop=(ki == 5))
                    a = psc.tile([128, 128], F16, tag="ndact", bufs=12,
                                 name=f"ndact{half}_{mo}")
                    tnd = psc.tile([128, 128], F32, tag="tmpnd", bufs=2)
                    nc.vector.tensor_scalar(out=tnd[:], in0=ps[:, :128],
                                            scalar1=SLOPE, scalar2=None, op0=ALU.mult)
                    nc.vector.tensor_tensor(out=a[:], in0=ps[:, :128], in1=tnd[:],
                                            op=ALU.max)
                    nd_act.append(a)
            # m1_nd = nd_act @ wnd_comb : [128 ent, 512] row-major
            ps_nd = pp.tile([128, 512], F32, tag="ps_mm", bufs=2, space="PSUM")
            for ki in range(12):
                wndc = pw.tile([128, 512], F16, tag="wndc", bufs=4)
                nc.sync.dma_start(out=wndc[:],
                                  in_=wnd_comb[128 * ki:128 * (ki + 1), :])
                nc.tensor.matmul(ps_nd[:], lhsT=nd_act[ki][:], rhs=wndc[:],
                                 start=(ki == 0), stop=(ki == 11))
            nd_loc = pmed.tile([128, 512], F16, tag="ndloc", bufs=1)
            nc.scalar.activation(nd_loc[:], ps_nd[:], ACTF.Copy)
            nc.sync.dma_start(out=agnd_in[:], in_=nd_loc[:])
            nc.gpsimd.collective_compute(
                "AllGather", ALU.bypass, replica_groups=RG,
                ins=[agnd_in[:]], outs=[agnd_out[:]])

            ndsb = [pkeep.tile([128, 512], F16, tag="ndsb", bufs=8,
                               name=f"ndsb{i}") for i in range(8)]
            for i in range(8):
                nc.sync.dma_start(out=ndsb[i][:],
                                  in_=agnd_out[128 * i:128 * (i + 1), :])
            if FP8_AG:
                nd8 = []
                for i in range(8):
                    n8 = pkeep.tile([128, 512], F8, tag="ndsb8", bufs=8,
                                    name=f"ndsb8_{i}")
                    nc.scalar.activation(n8[:], ndsb[i][:], ACTF.Copy,
                                         scale=AG_SCALE)
                    nd8.append(n8)
            else:
                nd8 = ndsb

            # ---- zpre: xt += (pre_c @ pre_W)^T (pre_b asserted zero) ----
            wpre = [pw.tile([128, 512], F16, tag="wres", bufs=16, name=f"wpre{k}")
                    for k in range(4)]
            wenc = [pw.tile([128, 512], F16, tag="wres", bufs=16, name=f"wenc{k}")
                    for k in range(4)]
            for k in range(4):
                nc.sync.dma_start(out=wpre[k][:], in_=pre_W[128 * k:128 * (k + 1), :])
                nc.sync.dma_start(out=wenc[k][:], in_=enc_W[128 * k:128 * (k + 1), :])
            for j in range(4):
                sl = slice(512 * j, 512 * (j + 1))
                pre_j = []
                for ki in range(4):
                    s = pmed.tile([128, 512], F16, tag="prestream", bufs=4)
                    nc.sync.dma_start(out=s[:], in_=pre_t_d[128 * ki:128 * (ki + 1), sl])
                    pre_j.append(s)
                for k in range(4):
                    ps = pp.tile([128, 512], F32, tag="ps_mm", bufs=2, space="PSUM")
                    for ki in range(4):
                        nc.tensor.matmul(ps[:], lhsT=wpre[ki][:, 128 * k:128 * (k + 1)],
                                         rhs=pre_j[ki][:], start=(ki == 0),
                                         stop=(ki == 3))
                    nc.vector.tensor_tensor(out=xt[k][:, sl], in0=xt[k][:, sl],
                                            in1=ps[:], op=ALU.add)

            # ---- gconv1 gather + one-hot PE scatter ----
            bounds1 = []
            for t_id, nch in enumerate(nch1_t):
                for j in range(nch):
                    bounds1.append((t_id, j == 0, j == nch - 1))
            sbufs = {}
            seg1 = []
            ps = None
            sel_r = sel1_d[:].rearrange("(c e) d -> e c d", e=128)
            for i in range(C1):
                w, slot = divmod(i, WAVE)
                if slot == 0:
                    nch_w = min(WAVE, C1 - w * WAVE)
                    assert nch_w == WAVE, "waves must divide chunk count"
                    if w >= PREPW:
                        g = pg.tile([128, WAVE, 512], GDT, tag="gath", bufs=4)
                        nc.gpsimd.dma_gather(
                            g[:, :nch_w, :], ag1_out[:],
                            idx1[:, WCOLS * w:WCOLS * w + nch_w * 8],
                            nch_w * 128, nch_w * 128, 512,
                            single_packet=True)
                        gbufs[w] = g
                    sw = pg.tile([128, WAVE, 128], GDT, tag="selw", bufs=4)
                    nc.sync.dma_start(
                        out=sw[:, :nch_w, :],
                        in_=sel_r[:, WAVE * w:WAVE * w + nch_w, :])
                    sbufs[w] = sw
                t_id, first, last = bounds1[i]
                if first:
                    ps = pp.tile([128, 512], F32, tag="ps_seg", bufs=2,
                                 space="PSUM")
                nc.tensor.matmul(ps[:], lhsT=sbufs[w][:, slot, :],
                                 rhs=gbufs[w][:, slot, :],
                                 start=first, stop=False)
                if last:
                    # fold in the per-entity nd contribution (neighbors+self)
                    ae_sb = pg.tile([128, 8, 128], GDT, tag="ae", bufs=3)
                    nc.sync.dma_start(
                        out=ae_sb[:],
                        in_=ae_d[128 * 8 * t_id:128 * 8 * (t_id + 1), :].rearrange(
                            "(e p) m -> p e m", p=128))
                    for e in range(8):
                        nc.tensor.matmul(ps[:], lhsT=ae_sb[:, e, :], rhs=nd8[e][:],
                                         start=False, stop=(e == 7))
                    seg1.append((t_id, ps))

            # ---- u = lrelu(seg + m1_self + ienc_b); xt += u^T; m2 per tile ----
            m2sb = [pkeep.tile([128, 512], F16, tag="m2keep", bufs=16,
                               name=f"m2sb{t}") for t in range(NT)]
            g2acc = []
            for q in range(4):
                for tt in range(4):
                    t_id, ps = seg1[4 * q + tt]
                    useg = pmed.tile([128, 512], F32, tag="useg", bufs=3)
                    nc.vector.tensor_tensor(out=useg[:], in0=ps[:],
                                            in1=m1sb[t_id][:], op=ALU.add)
                    tmp = pmed.tile([128, 512], F32, tag="lrtmp", bufs=2)
                    ua = pmed.tile([128, 512], F16, tag="ua", bufs=3)
                    nc.vector.tensor_scalar(out=tmp[:], in0=useg[:], scalar1=SLOPE,
                                            scalar2=None, op0=ALU.mult)
                    nc.vector.tensor_tensor(out=ua[:], in0=useg[:], in1=tmp[:],
                                            op=ALU.max)
                    pst = pp.tile([128, 512], F16, tag="ps_ut", bufs=2, space="PSUM")
                    for k in range(4):
                        nc.tensor.transpose(
                            out=pst[:, 128 * k:128 * (k + 1)],
                            in_=ua[:, 128 * k:128 * (k + 1)], identity=ident16[:])
                    csl = slice(512 * q + 128 * tt, 512 * q + 128 * (tt + 1))
                    for k in range(4):
                        nc.vector.tensor_tensor(
                            out=xt[k][:, csl], in0=xt[k][:, csl],
                            in1=pst[:, 128 * k:128 * (k + 1)], op=ALU.add)
                for tt in range(4):
                    t = 4 * q + tt
                    tsl = slice(128 * t, 128 * (t + 1))
                    ps = pp.tile([128, 512], F32, tag="ps_mm", bufs=2, space="PSUM")
                    for ki in range(4):
                        nc.tensor.matmul(ps[:], lhsT=xt[ki][:, tsl], rhs=wenc[ki][:],
                                         start=(ki == 0), stop=(ki == 3))
                    nc.scalar.activation(m2sb[t][:], ps[:], ACTF.Copy)
                    if DEBUG:
                        nc.sync.dma_start(out=dbg_m2[tsl, :], in_=m2sb[t][:])
                if q == 3:
                    # pass over t12-14 right away; only t15 remains for the
                    # tiny final pass
                    for s in range(8):
                        a2sb = pg.tile([128, 3, 128], F16, tag="a2q3", bufs=3)
                        nc.sync.dma_start(
                            out=a2sb[:],
                            in_=a2_d[128 * (16 * s + 12):128 * (16 * s + 15),
                                     :].rearrange("(t p) m -> p t m", p=128))
                        psg = pp.tile([128, 512], F32, tag="ps_mm", bufs=2,
                                      space="PSUM")
                        for t in range(12, 15):
                            nc.tensor.matmul(psg[:], lhsT=a2sb[:, t - 12, :],
                                             rhs=m2sb[t][:],
                                             start=(t == 12), stop=(t == 14))
                        nc.vector.tensor_tensor(out=g2acc[s][:], in0=g2acc[s][:],
                                                in1=psg[:], op=ALU.add)
                if q in (1, 2):
                    # gconv2 partial passes: accumulate ready m2 tiles while
                    # later gather waves are still in flight
                    t0, t1 = (0, 8) if q == 1 else (8, 12)
                    nacc = []
                    for s in range(8):
                        a2sb = pg.tile([128, t1 - t0, 128], F16,
                                       tag=f"a2p{q}", bufs=3)
                        nc.sync.dma_start(
                            out=a2sb[:],
                            in_=a2_d[128 * (16 * s + t0):128 * (16 * s + t1),
                                     :].rearrange("(t p) m -> p t m", p=128))
                        psg = pp.tile([128, 512], F32, tag="ps_mm", bufs=2,
                                      space="PSUM")
                        for t in range(t0, t1):
                            nc.tensor.matmul(psg[:], lhsT=a2sb[:, t - t0, :],
                                             rhs=m2sb[t][:],
                                             start=(t == t0), stop=(t == t1 - 1))
                        if q == 1:
                            ac = pmed.tile([128, 512], F16, tag="g2acc", bufs=8)
                            nc.scalar.activation(ac[:], psg[:], ACTF.Copy)
                            nacc.append(ac)
                        else:
                            nc.vector.tensor_tensor(out=g2acc[s][:],
                                                    in0=g2acc[s][:],
                                                    in1=psg[:], op=ALU.add)
                    if q == 1:
                        g2acc = nacc

            # ---- gconv2 final pass: only m2 tile 15 + accumulator + RS ----
            for s in range(8):
                a2sb = pg.tile([128, 1, 128], F16, tag="a2b", bufs=3)
                nc.sync.dma_start(
                    out=a2sb[:],
                    in_=a2_d[128 * (16 * s + 15):128 * 16 * (s + 1), :].rearrange(
                        "(t p) m -> p t m", p=128))
                ps2 = pp.tile([128, 512], F32, tag="ps_seg", bufs=2, space="PSUM")
                nc.tensor.matmul(ps2[:], lhsT=a2sb[:, 0, :], rhs=m2sb[15][:],
                                 start=True, stop=True)
                pc = pmed.tile([128, 512], F16, tag="rsc", bufs=3)
                nc.vector.tensor_tensor(out=pc[:], in0=ps2[:], in1=g2acc[s][:],
                                        op=ALU.add)
                nc.sync.dma_start(out=rs_in[128 * s:128 * (s + 1), :], in_=pc[:])
            nc.gpsimd.collective_compute(
                "ReduceScatter", ALU.add, replica_groups=RG,
                ins=[rs_in[:]], outs=[rs_out[:]])

            # ---- zk^T + readout ----
            zk16 = pmed.tile([128, 512], F16, tag="zk16", bufs=1)
            nc.sync.dma_start(out=zk16[:], in_=rs_out[:])
            if DEBUG:
                nc.sync.dma_start(out=dbg_zk[:], in_=zk16[:])
            ps_zt = pp.tile([128, 512], F16, tag="ps_ut", bufs=2, space="PSUM")
            for k in range(4):
                nc.tensor.transpose(out=ps_zt[:, 128 * k:128 * (k + 1)],
                                    in_=zk16[:, 128 * k:128 * (k + 1)],
                                    identity=ident16[:])
            zkt = pmed.tile([128, 512], F16, tag="zkt", bufs=1)
            for k in range(4):
                sl = slice(128 * k, 128 * (k + 1))
                tmp = pmed.tile([128, 128], F32, tag="lrtmp2", bufs=2)
                nc.vector.tensor_scalar(
                    out=tmp[:], in0=ps_zt[:, sl], scalar1=SLOPE, scalar2=None,
                    op0=ALU.mult)
                nc.vector.tensor_tensor(
                    out=zkt[:, sl], in0=ps_zt[:, sl], in1=tmp[:], op=ALU.max)

            wg1 = [pw.tile([128, 512], F16, tag="wres32", bufs=4, name=f"wg1{k}")
                   for k in range(4)]
            for k in range(4):
                nc.sync.dma_start(out=wg1[k][:], in_=gate_W1[128 * k:128 * (k + 1), :])
            w2r = psc.tile([128, 8], F16, tag="w2r", bufs=1)
            nc.sync.dma_start(out=w2r[:], in_=gw2reg[:])
            s1t = pmed.tile([128, 512], F16, tag="s1t", bufs=1)
            for ko_ in range(4):
                ps = pp.tile([128, 512], F32, tag="ps_mm", bufs=2, space="PSUM")
                for ki in range(4):
                    nc.tensor.matmul(ps[:, :128],
                                     lhsT=wg1[ki][:, 128 * ko_:128 * (ko_ + 1)],
                                     rhs=zkt[:, 128 * ki:128 * (ki + 1)],
                                     start=(ki == 0), stop=False)
                nc.tensor.matmul(ps[:, :128],
                                 lhsT=brow_g[:, 128 * ko_:128 * (ko_ + 1)],
                                 rhs=ones[:, :128], start=False, stop=True)
                nc.scalar.activation(s1t[:, 128 * ko_:128 * (ko_ + 1)], ps[:, :128],
                                     ACTF.Tanh)
            ps_sc = pp.tile([128, 512], F32, tag="ps_mm", bufs=2, space="PSUM")
            for ki in range(4):
                nc.tensor.matmul(ps_sc[:1, :128], lhsT=w2r[:, 2 * ki:2 * ki + 1],
                                 rhs=s1t[:, 128 * ki:128 * (ki + 1)],
                                 start=(ki == 0), stop=(ki == 3))
            ps_tr = pp.tile([128, 512], F32, tag="ps_seg", bufs=2, space="PSUM")
            for ki in range(4):
                nc.tensor.matmul(ps_tr[:1, :128], lhsT=w2r[:, 2 * ki + 1:2 * ki + 2],
                                 rhs=zkt[:, 128 * ki:128 * (ki + 1)],
                                 start=(ki == 0), stop=(ki == 3))
            erow = psc.tile([1, 128], F32, tag="erow", bufs=1)
            nc.scalar.activation(erow[:], ps_sc[:1, :128], ACTF.Exp,
                                 bias=bpf[:1, 24:25])
            etrow = psc.tile([1, 128], F32, tag="etrow", bufs=1)
            nc.vector.tensor_tensor(out=etrow[:], in0=erow[:], in1=ps_tr[:1, :128],
                                    op=ALU.mult)
            sums = psc.tile([1, 4], F32, tag="sums", bufs=1)
            nc.vector.tensor_reduce(out=sums[:, 0:2],
                                    in_=etrow[:].rearrange("p (g x) -> p g x", g=2),
                                    axis=AX, op=ALU.add)
            nc.vector.tensor_reduce(out=sums[:, 2:4],
                                    in_=erow[:].rearrange("p (g x) -> p g x", g=2),
                                    axis=AX, op=ALU.add)
            res = psc.tile([1, 4], F32, tag="res", bufs=1)
            nc.vector.reciprocal(out=res[:, 2:4], in_=sums[:, 2:4])
            nc.vector.tensor_tensor(out=res[:, 0:2], in0=sums[:, 0:2],
                                    in1=res[:, 2:4], op=ALU.mult)
            nc.vector.tensor_scalar(out=res[:, 0:2], in0=res[:, 0:2],
                                    scalar1=bpf[:1, 25:26], scalar2=None, op0=ALU.add)
            nc.sync.dma_start(out=out_d[:], in_=res[:, 0:2])

    nc.compile()
    return nc


def _ensure_ntff_hook():
    """Inject antenv.axon_hooks (absent in this image) so trace=True works."""
    import sys, types
    try:
        from antenv.axon_hooks import get_axon_ntff_profile_hook  # noqa
        return
    except ImportError:
        pass
    import antenv
    mod = types.ModuleType("antenv.axon_hooks")
    _state = {"hook": None}
    mod.set_axon_ntff_profile_hook = lambda h: _state.__setitem__("hook", h)
    mod.get_axon_ntff_profile_hook = lambda: _state["hook"]
    sys.modules["antenv.axon_hooks"] = mod
    antenv.axon_hooks = mod
    from trn_agent_boot.trn_boot import _ntff_profile_via_ctypes
    mod.set_axon_ntff_profile_hook(
        _ntff_profile_via_ctypes("/opt/axon/libaxon_pjrt.so"))


# ---------------------------------------------------------------------------
# host wrapper
# ---------------------------------------------------------------------------

def kernel(**inputs):
    f32 = lambda k: np.asarray(inputs[k], np.float32)
    x = f32("x"); pre_x = f32("pre_x")
    edge_index = np.asarray(inputs["edge_index"], np.int64)
    internal_edge_index = np.asarray(inputs["internal_edge_index"], np.int64)
    name_emb = f32("name_embeddings"); desc_emb = f32("desc_embeddings")
    ko_mask = np.asarray(inputs["ko_mask"], np.int64)
    bkm = np.asarray(inputs["batch_ko_masks"], np.int64)
    name_W = f32("name_W"); name_b = f32("name_b")
    desc_W = f32("desc_W"); desc_b = f32("desc_b")
    omic_W = f32("omic_W"); omic_b = f32("omic_b")
    fus_W = f32("fus_W"); fus_b = f32("fus_b")
    pre_W = f32("pre_W"); pre_b = f32("pre_b")
    ienc_W = f32("ienc_W"); ienc_b = f32("ienc_b")
    enc_W = f32("enc_W"); enc_b = f32("enc_b")
    gate_W1 = f32("gate_W1"); gate_b1 = f32("gate_b1")
    gate_W2 = f32("gate_W2"); gate_b2 = f32("gate_b2")
    reg_W = f32("reg_W"); reg_b = f32("reg_b")

    assert not fus_b.any() and not pre_b.any(), \
        "nonzero fus_b/pre_b not supported by this build"
    for bname, bv in [("name_b", name_b), ("desc_b", desc_b), ("omic_b", omic_b),
                      ("ienc_b", ienc_b), ("enc_b", enc_b)]:
        assert not bv.any(), f"nonzero {bname} not supported by this build"

    ko_feat = np.zeros(N, np.float32)
    ko_feat[ko_mask] = 1.0

    # fused weights
    f16 = np.float16
    fus_nd = fus_W[:2 * TX]
    fus_om = fus_W[2 * TX:]
    fus_om_pad = _pad_w(fus_om, 512, 512)
    w_comb = (fus_om_pad.astype(f16).astype(np.float32)
              @ ienc_W.astype(f16).astype(np.float32))
    w_comb[511, :] = ienc_W[511, :]
    wnd_comb = (fus_nd.astype(f16).astype(np.float32)
                @ ienc_W[:511].astype(f16).astype(np.float32))

    # ---- per-core local row permutation: balance in-degree across the 16
    # row tiles so every tile needs exactly ceil(avg) gather chunks ----
    s1_all, d1_all = internal_edge_index[0], internal_edge_index[1]
    perm_pos = []    # perm_pos[c][old_local] = new_local
    old_of = []      # old_of[c][new_local] = old_local
    for c in range(NCORE):
        lo, hi = R * c, R * (c + 1)
        m = (d1_all >= lo) & (d1_all < hi)
        deg = np.bincount(d1_all[m] - lo, minlength=R)
        order = np.argsort(-deg, kind="stable")
        loads = np.zeros(NT, np.int64)
        counts = np.zeros(NT, np.int64)
        assign = np.zeros(R, np.int64)
        for r_ in order:
            open_t = np.nonzero(counts < 128)[0]
            t_ = open_t[np.argmin(loads[open_t])]
            assign[r_] = t_
            loads[t_] += deg[r_]
            counts[t_] += 1
        # push any excess above 1024 into tile 15 via row swaps, so only
        # tile 15's chunk count can exceed ceil(1024/128)
        tile_rows = [list(np.nonzero(assign == t_)[0]) for t_ in range(NT)]
        for _ in range(512):
            over = [t_ for t_ in range(NT - 1) if loads[t_] > 1024]
            if not over:
                break
            t_ = max(over, key=lambda u: loads[u])
            need = int(loads[t_] - 1024)
            r2 = min(tile_rows[NT - 1], key=lambda r: deg[r])
            cands = [r for r in tile_rows[t_] if deg[r] > deg[r2]]
            if not cands:
                break
            # smallest swap that clears the excess; else the biggest available
            good = [r for r in cands if deg[r] - deg[r2] >= need]
            r1 = (min(good, key=lambda r: deg[r]) if good
                  else max(cands, key=lambda r: deg[r]))
            dd = deg[r1] - deg[r2]
            tile_rows[t_].remove(r1); tile_rows[t_].append(r2)
            tile_rows[NT - 1].remove(r2); tile_rows[NT - 1].append(r1)
            loads[t_] -= dd; loads[NT - 1] += dd
            assign[r1], assign[r2] = NT - 1, t_
        pos = np.zeros(R, np.int64)
        nxt = np.zeros(NT, np.int64)
        for r_ in range(R):
            t_ = assign[r_]
            pos[r_] = 128 * t_ + nxt[t_]
            nxt[t_] += 1
        perm_pos.append(pos)
        old_of.append(np.argsort(pos, kind="stable"))

    # global old row -> global "new" position (within its core block)
    gnew = np.concatenate([R * c + perm_pos[c] for c in range(NCORE)])

    per_core_1 = []
    nch1_t = np.ones(NT, np.int64)
    for c in range(NCORE):
        lo, hi = R * c, R * (c + 1)
        m = (d1_all >= lo) & (d1_all < hi)
        per_core_1.append((gnew[s1_all[m]], perm_pos[c][d1_all[m] - lo]))
        cnt = np.bincount(perm_pos[c][d1_all[m] - lo] >> 7, minlength=NT)
        nch1_t = np.maximum(nch1_t, -(-cnt // 128))
    pad1 = (-int(nch1_t.sum())) % WAVE
    nch1_t[NT - 1] += pad1
    nch1_t = tuple(int(v) for v in nch1_t)

    # ---- gconv1 nd-side: per-entity count matrices (neighbors + self) ----
    ae_percore = []
    ent_of = np.arange(N, dtype=np.int64) % NE
    for c in range(NCORE):
        lo, hi = R * c, R * (c + 1)
        AE = np.zeros((16, 8, 128, 128), np.float32)
        m = (d1_all >= lo) & (d1_all < hi)
        uu = s1_all[m]; vv = perm_pos[c][d1_all[m] - lo]
        ee = ent_of[uu]
        np.add.at(AE, (vv >> 7, ee >> 7, ee & 127, vv & 127), 1.0)
        rr = np.arange(R, dtype=np.int64)
        es = ent_of[lo + old_of[c][rr]]
        np.add.at(AE, (rr >> 7, es >> 7, es & 127, rr & 127), 1.0)
        ae_percore.append(AE)

    # ---- gconv2: block-dense count matrices into the 1024 KO slots ----
    slot_row = (bkm + np.arange(B)[:, None] * NE).reshape(-1)   # [1024]
    row2slots = {}
    for s_, r_ in enumerate(slot_row):
        row2slots.setdefault(int(r_), []).append(s_)
    s2_all, d2_all = edge_index[0], edge_index[1]
    m2mask = np.isin(d2_all, slot_row)
    a2_percore = []
    for c in range(NCORE):
        lo, hi = R * c, R * (c + 1)
        A2 = np.zeros((8, 16, 128, 128), np.float32)
        for s_, r_ in enumerate(slot_row):
            if lo <= r_ < hi:
                rl = int(perm_pos[c][int(r_) - lo])
                A2[s_ >> 7, rl >> 7, rl & 127, s_ & 127] += 1.0
        mm = m2mask & (s2_all >= lo) & (s2_all < hi)
        for u, v in zip(s2_all[mm], d2_all[mm]):
            rl = int(perm_pos[c][int(u) - lo])
            for s_ in row2slots[int(v)]:
                A2[s_ >> 7, rl >> 7, rl & 127, s_ & 127] += 1.0
        a2_percore.append(np.ascontiguousarray(
            A2.reshape(128 * 128, 128)).astype(f16))

    nc = _build(nch1_t)

    omic_Wp = _pad_w(omic_W, 512, 512)
    bias_pf = np.zeros((128, 26), np.float32)
    bias_pf[:, 0:6] = name_b.reshape(6, 128).T
    bias_pf[:, 6:12] = desc_b.reshape(6, 128).T
    bias_pf[:, 12:16] = _pad_w(omic_b[:, None], 512, 1).reshape(4, 128).T
    bias_pf[:, 16:20] = ienc_b.reshape(4, 128).T
    bias_pf[:, 20:24] = enc_b.reshape(4, 128).T
    bias_pf[:, 24] = float(gate_b2.reshape(-1)[0])
    bias_pf[:, 25] = float(reg_b.reshape(-1)[0])
    bias_rows = np.zeros((96, 512), np.float16)
    bias_rows[64, :] = gate_b1.astype(f16)
    gw2 = np.concatenate([gate_W2, reg_W], axis=1).astype(f16)
    gw2 = np.ascontiguousarray(
        gw2.reshape(4, 128, 2).transpose(1, 0, 2).reshape(128, 8))

    sel_dt = ml_dtypes.float8_e4m3fn if FP8_AG else f16
    shared = dict(
        name_W=name_W.astype(f16), desc_W=desc_W.astype(f16),
        wnd_comb=wnd_comb.astype(f16), omic_W=omic_Wp.astype(f16),
        w_comb=w_comb.astype(f16), pre_W=pre_W.astype(f16),
        enc_W=enc_W.astype(f16),
        gate_W1=gate_W1.astype(f16), gw2reg=gw2, bias_pf=bias_pf,
        bias_rows=bias_rows,
    )

    in_maps = []
    for c in range(NCORE):
        lo, hi = R * c, R * (c + 1)
        rows = lo + old_of[c]
        x_t = np.concatenate([x[rows].T, ko_feat[None, rows]], 0)
        pre_t = np.concatenate([pre_x[rows].T, ko_feat[None, rows]], 0)
        ndemb = np.concatenate(
            [name_emb[128 * c:128 * (c + 1)].T, desc_emb[128 * c:128 * (c + 1)].T], 0)
        i1, dv1 = _chunk_edges_per_tile(*per_core_1[c], nch1_t)
        in_maps.append(dict(
            x_t=np.ascontiguousarray(x_t).astype(f16),
            pre_t=np.ascontiguousarray(pre_t).astype(f16),
            ndemb=np.ascontiguousarray(ndemb).astype(f16),
            idx1=_wrap_idx_waves(i1),
            sel1=_sel_from_dstv(dv1, sel_dt,
                                1.0 / AG_SCALE if FP8_AG else 1.0),
            ae=np.ascontiguousarray(
                ae_percore[c] * (1.0 / AG_SCALE if FP8_AG else 1.0)
            ).reshape(16 * 8 * 128, 128).astype(sel_dt),
            a2=a2_percore[c],
            **shared,
        ))

    if TRACE:
        _ensure_ntff_hook()
    res = run_bass_kernel_spmd(nc, in_maps, core_ids=list(range(NCORE)),
                               trace=TRACE, **(TRACE_KW or {}))
    kernel._last = res
    out = np.zeros(B, np.float32)
    for c in range(NCORE):
        out[2 * c:2 * c + 2] = res.results[c]["out"][0]
    return out
